# revision 39
# baseline (speedup 1.0000x reference)
"""Trainium2 Bass kernel for nn_MixedFrequencyAttention.

Sharding: spatial over the H (rows) axis of the 48x48 image — 6 query rows
per core, with a 12-row K/V halo window. The 1x1 conv + BN + ReLU tail is
pointwise in space, so no cross-core communication is needed.

Attention is blocked by query-row PAIRS: each pair of query rows (x, x+1)
attends within an 8-kv-row window (384 tokens = 3 chunks of 128). The host
ships the kv activations pre-gathered per pair window (with row duplication)
so all SBUF offsets are core-independent (SPMD). This cuts the masked-dense
P/exp/mask work 1.67x vs a full 12-row halo and shrinks PSUM working set to
2 banks per head (double-buffered).

Per-branch device pipeline (channel-on-partition "transposed" layouts):
  Q^T = Wq^T X_q^T     [256(h,d), 576]     (bias via activation)
  K^T = Wk^T X_kvp^T   [256(h,d), 2304]    (pair-gathered kv tokens)
  V   = X_kvp Wv       [2304, 8*(32+32)]   (per-head ones cols -> denominator;
                                            V bias folded into O-proj bias)
  S[kv, q] = K_chunk^T Q   per (b, h, pair, chunk)   [128, 96] in PSUM
  P = exp(scale * S) * E   (E: exp(rpb) in band else 0, bf16)
  av[64, 96] = sum_c V_chunk^T P      (rows 32:63 = replicated denominator)
  attn = av[:32] * 1/av[32:64]        (DVE reciprocal + multiply)
  refine^T = Wo^T attn (+bo')  -> concat sa|ca  [512, 576]
  y[t, o] = refine^T_chunks^T @ Wconv' (+folded BN bias), ReLU

All matmul feeds are bf16; PSUM accumulation stays fp32.
"""

import math
import os
import sys

import numpy as np

sys.path.insert(0, "/opt/trn_rl_repo")

import ml_dtypes

B = 2
HS = 48
C = 512
C2 = 256
HEADS = 8
D = 32
KW = 7
BN_EPS = 1e-5

NX = 6          # query rows per core
NKV = 12        # kv halo rows per core
NP = 3          # query-row pairs per core
WR = 8          # kv rows per pair window
WT = WR * HS    # 384 kv tokens per pair window
NQ = NX * HS    # 288 queries per batch per core
NPQ = 2 * HS    # 96 queries per pair per batch
NT = B * NQ     # 576 query tokens per core
NKVP = B * NP * WT   # 2304 pair-gathered kv tokens per core
NCH = NKVP // 128    # 18 V chunks
SCALE = D ** -0.5
N_CORES = 8

# 9 (pair, chunk) S-blocks of 96 queries; blocks 0-4 live in PSUM bank 0
# (offsets 0..480), blocks 5-8 in bank 1 (offsets 512..896). The exp
# evacuation packs them contiguously at idx*96 in SBUF.
def _ps_off(idx):
    return idx * 96 if idx < 5 else 512 + (idx - 5) * 96


F32 = np.float32
BF16 = ml_dtypes.bfloat16


def _s_clip(g):
    return np.clip(g - 3, 0, HS - KW)


def _chunk128(a):
    """[256, N] -> [128, 2*N] chunk-major free layout."""
    n = a.shape[1]
    return a.reshape(2, 128, n).transpose(1, 0, 2).reshape(128, 2 * n)


def _chunk128_4(a):
    n = a.shape[1]
    return a.reshape(4, 128, n).transpose(1, 0, 2).reshape(128, 4 * n)


def _core_geometry(c):
    q_lo = NX * c
    kv_lo = int(np.clip(q_lo - 3, 0, HS - NKV))
    ls = _s_clip(np.arange(q_lo, q_lo + NX)) - kv_lo  # [6], in [0, 5]
    rw = np.minimum(ls[0::2], NKV - WR)               # [3] pair window starts
    return q_lo, kv_lo, ls, rw


def _build_E(c, rpb):
    """Mask/bias tensor, [128, 8 * 864] (p, h*864 + (pair*3+chunk)*96 + q96)."""
    q_lo, kv_lo, ls, rw = _core_geometry(c)
    E = np.zeros((128, HEADS, NP, 3, NPQ), dtype=F32)

    sy = _s_clip(np.arange(HS))  # [48]
    p = np.arange(128)
    for j in range(NP):
        for cb in range(3):
            kv = cb * 128 + p
            r = rw[j] + kv // HS     # [128] local kv row in halo
            ky = kv % HS             # [128]
            for xi in range(2):
                x = 2 * j + xi
                u = r - ls[x]
                row_live = (u >= 0) & (u < KW)                 # [128]
                ri = np.clip(kv_lo + r - (q_lo + x) + KW - 1, 0, 2 * KW - 2)
                y = np.arange(HS)
                v = ky[:, None] - sy[None, :]
                col_live = (v >= 0) & (v < KW)                 # [128, 48]
                rj = np.clip(ky[:, None] - y[None, :] + KW - 1, 0, 2 * KW - 2)
                live = row_live[:, None] & col_live
                for h in range(HEADS):
                    vals = np.exp(rpb[h][ri[:, None], rj])     # [128, 48]
                    E[:, h, j, cb, xi * HS:(xi + 1) * HS] = \
                        np.where(live, vals, 0.0)
    return E.reshape(128, HEADS * NP * 3 * NPQ)


def _fold_conv(conv_w, conv_b, bn_gamma, bn_beta, bn_mean, bn_var):
    s = bn_gamma / np.sqrt(bn_var + BN_EPS)
    Wp = (conv_w * s[:, None]).T.astype(F32)          # [512 c_in, 512 o]
    bp = (conv_b * s + bn_beta - bn_mean * s).astype(F32)  # [512]
    return Wp, bp


def _prep_core(inputs, c):
    """Host-side shard/transform for core c -> dict of DRAM input arrays."""
    q_lo, kv_lo, ls, rw = _core_geometry(c)
    sal = np.ascontiguousarray(inputs["sal_feat"]).reshape(B, HS, HS, C)
    edge = np.ascontiguousarray(inputs["edge_feat"])  # [B, 256, 48, 48]

    xq_sa = np.ascontiguousarray(
        sal[:, q_lo:q_lo + NX, :, :C2].transpose(3, 0, 1, 2).reshape(C2, NT)
    ).astype(F32)
    xq_ca = np.ascontiguousarray(
        edge[:, :, q_lo:q_lo + NX, :].transpose(1, 0, 2, 3).reshape(C2, NT)
    ).astype(F32)

    def kv_pairs(a4):  # [B, 48, 48, 256] full rows -> [256, B*3*384]
        blocks = []
        for b in range(B):
            for j in range(NP):
                r0 = kv_lo + rw[j]
                blocks.append(a4[b, r0:r0 + WR, :, :].reshape(WT, C2))
        return np.ascontiguousarray(
            np.concatenate(blocks, 0).T).astype(F32)   # [256, 2304]

    xkv_sa = kv_pairs(sal[:, :, :, :C2])
    xkv_ca = kv_pairs(sal[:, :, :, C2:])

    def wblob(pfx):
        wq = inputs[pfx + "_wq"].astype(F32)
        wkv = inputs[pfx + "_wkv"].astype(F32)
        wo = inputs[pfx + "_wo"].astype(F32)
        blob = np.concatenate(
            [_chunk128(wq), _chunk128(wkv), _chunk128(wo)], axis=1)  # [128,2048]
        bq = inputs[pfx + "_bq"].astype(np.float64)
        bk = inputs[pfx + "_bkv"].astype(np.float64)[:C2]
        bv = inputs[pfx + "_bkv"].astype(np.float64)[C2:]
        # V bias folded into the O-projection bias: attn' = attn + bv
        # => refine = Wo^T attn' + bo = Wo^T attn + (bo + Wo^T bv)
        bo = (inputs[pfx + "_bo"].astype(np.float64)
              + wo.astype(np.float64).T @ bv)
        b2 = np.stack([bq[:128], bq[128:], bk[:128], bk[128:],
                       bo[:128], bo[128:]], axis=1).astype(F32)  # [128, 6]
        return blob, b2

    w_sa, b2_sa = wblob("sa")
    w_ca, b2_ca = wblob("ca")
    Wp, bp = _fold_conv(
        inputs["conv_w"].astype(np.float64), inputs["conv_b"].astype(np.float64),
        inputs["bn_gamma"].astype(np.float64), inputs["bn_beta"].astype(np.float64),
        inputs["bn_mean"].astype(np.float64), inputs["bn_var"].astype(np.float64))

    def bf(a):
        return np.ascontiguousarray(a).astype(BF16)

    return {
        "xq_sa": bf(_chunk128(xq_sa)), "xkv_sa": bf(_chunk128(xkv_sa)),
        "xq_ca": bf(_chunk128(xq_ca)), "xkv_ca": bf(_chunk128(xkv_ca)),
        "w_sa": bf(w_sa), "w_ca": bf(w_ca),
        "b2_sa": b2_sa.astype(F32), "b2_ca": b2_ca.astype(F32),
        "w_conv": bf(_chunk128_4(Wp)), "b_conv": bf(bp[None, :]),
        "e_sa": bf(_build_E(c, inputs["sa_rpb"].astype(F32))),
        "e_ca": bf(_build_E(c, inputs["ca_rpb"].astype(F32))),
    }


# ---------------------------------------------------------------------------
# Pure-numpy mirror of the device program (for validating the decomposition)
# ---------------------------------------------------------------------------

def _mirror_core(ci):
    def unchunk(a, k):  # [128, k*n] -> [128k, n]
        n = a.shape[1] // k
        return a.reshape(128, k, n).transpose(1, 0, 2).reshape(128 * k, n)

    def branch(xq, xkv, w, b2, e):
        w = np.asarray(w, F32)
        xq = np.asarray(xq, F32)
        xkv = np.asarray(xkv, F32)
        e = np.asarray(e, F32)
        wq = unchunk(w[:, :512], 2)        # [256, 256]
        wkv = unchunk(w[:, 512:1536], 2)   # [256, 512]
        wo = unchunk(w[:, 1536:], 2)       # [256, 256]
        bq = np.concatenate([b2[:, 0], b2[:, 1]])
        bk = np.concatenate([b2[:, 2], b2[:, 3]])
        bo = np.concatenate([b2[:, 4], b2[:, 5]])
        XqT = unchunk(xq, 2)               # [256, 576]
        XkvT = unchunk(xkv, 2)             # [256, 2304]
        QT = wq.T @ XqT + bq[:, None]      # [256, 576]
        KT = wkv[:, :C2].T @ XkvT + bk[:, None]   # [256, 2304]
        V = XkvT.T @ wkv[:, C2:]           # [2304, 256] (no bias)
        Emask = e.reshape(128, HEADS, NP, 3, NPQ)

        attn = np.zeros((C2, NT), dtype=F32)
        for b in range(B):
            for h in range(HEADS):
                for j in range(NP):
                    Q_h = QT[32 * h:32 * h + 32,
                             b * NQ + j * NPQ:b * NQ + (j + 1) * NPQ]
                    out = np.zeros((64, NPQ), dtype=F32)
                    for cb in range(3):
                        k0 = (b * NP + j) * WT + cb * 128
                        Kc = KT[32 * h:32 * h + 32, k0:k0 + 128]
                        S = Kc.T @ Q_h                       # [128, 96]
                        P = np.exp(SCALE * S) * Emask[:, h, j, cb, :]
                        Vc = V[k0:k0 + 128, 32 * h:32 * h + 32]
                        Vaug = np.concatenate(
                            [Vc, np.ones((128, 32), F32)], axis=1)  # [128,64]
                        out += Vaug.T @ P
                    attn[32 * h:32 * h + 32,
                         b * NQ + j * NPQ:b * NQ + (j + 1) * NPQ] = \
                        out[:32] / out[32:]
        return wo.T @ attn + bo[:, None]   # [256, 576]

    r_sa = branch(ci["xq_sa"], ci["xkv_sa"], ci["w_sa"], ci["b2_sa"], ci["e_sa"])
    r_ca = branch(ci["xq_ca"], ci["xkv_ca"], ci["w_ca"], ci["b2_ca"], ci["e_ca"])
    refine = np.concatenate([r_sa, r_ca], axis=0)  # [512, 576]
    Wc = np.asarray(ci["w_conv"], F32)
    Wc = Wc.reshape(128, 4, 512).transpose(1, 0, 2).reshape(512, 512)
    y = refine.T @ Wc + np.asarray(ci["b_conv"], F32)   # [576, 512]
    return np.maximum(y, 0.0)                      # [t, o]


def mirror(inputs):
    """Full-output numpy mirror: returns [B, 512, 48, 48]."""
    out = np.zeros((B, C, HS, HS), dtype=F32)
    for c in range(N_CORES):
        ci = _prep_core(inputs, c)
        y = _mirror_core(ci)  # [576, 512]
        q_lo = NX * c
        yb = y.reshape(B, NX, HS, C).transpose(0, 3, 1, 2)  # [B, 512, 6, 48]
        out[:, :, q_lo:q_lo + NX, :] = yb
    return out


# ---------------------------------------------------------------------------
# Bass program
# ---------------------------------------------------------------------------


def _patch_tile_tail():
    """This container's walrus rejects instructions carrying more than ~1
    sync-wait ("Too many sync wait commands" on the Tile tail drain).
    Split the tail's global-clock waits across per-proc NOPs on the sync
    engine so each instruction carries at most one wait."""
    import concourse.tile as tile_mod
    from concourse.vector_clock import ScopedClock, VectorClock

    if getattr(tile_mod.TileContext, "_tail_patched", False):
        return

    def _drain_and_barrier(self, tick_clock, wait_clock):
        gc = tick_clock.global_clock
        n = len(gc)
        for p in range(n):
            if gc[p] == 0:
                continue
            partial = VectorClock([gc[i] if i == p else 0 for i in range(n)])
            ni = self.nc.sync.nop()
            wait_clock.add_sem_waits(ni.ins, ScopedClock({None: partial}))
        self.nc.sync.drain()
        self.nc.all_engine_barrier()
        assert self.sems is not None
        popped = self.nc._tile_sem_poison_stack.pop()
        assert popped is self._sem_poison
        self.nc.clear_and_free_semaphores(list(self.sems.allocated().values()))
        self.nc.all_engine_barrier()

    tile_mod.TileContext._drain_and_barrier = _drain_and_barrier
    tile_mod.TileContext._tail_patched = True


def build_nc(mm_dtype_name="bfloat16", split_waits=True):
    import concourse.bass as bass
    import concourse.mybir as mybir
    from concourse.tile import TileContext

    _patch_tile_tail()

    mm_dt = getattr(mybir.dt, mm_dtype_name)
    f32 = mybir.dt.float32
    af = mm_dt  # dtype for every tensor that feeds a matmul

    nc = bass.Bass()

    def din(name, shape, dt=None):
        return nc.dram_tensor(name, shape, dt or af, kind="ExternalInput")

    xq = {"sa": din("xq_sa", [128, 2 * NT]), "ca": din("xq_ca", [128, 2 * NT])}
    xkv = {"sa": din("xkv_sa", [128, 2 * NKVP]),
           "ca": din("xkv_ca", [128, 2 * NKVP])}
    wb = {"sa": din("w_sa", [128, 2048]), "ca": din("w_ca", [128, 2048])}
    bb2 = {"sa": din("b2_sa", [128, 6], f32), "ca": din("b2_ca", [128, 6], f32)}
    eb = {"sa": din("e_sa", [128, 6912]), "ca": din("e_ca", [128, 6912])}
    wconv = din("w_conv", [128, 2048])
    bconv = din("b_conv", [1, 512])
    y_out = nc.dram_tensor("y", [128, 5 * C], f32, kind="ExternalOutput")

    def mm(ap):
        return ap

    with TileContext(nc) as tc:
        import contextlib
        ctx = contextlib.ExitStack()
        with ctx:
            sb = ctx.enter_context(tc.tile_pool(name="sb", bufs=1))
            sbP = ctx.enter_context(tc.tile_pool(name="sbP", bufs=4))
            sbV = ctx.enter_context(tc.tile_pool(name="sbV", bufs=4))
            pp = ctx.enter_context(
                tc.tile_pool(name="pp", bufs=2, space="PSUM"))
            ppS = ctx.enter_context(
                tc.tile_pool(name="ppS", bufs=2, space="PSUM"))
            ppAV = ctx.enter_context(
                tc.tile_pool(name="ppAV", bufs=2, space="PSUM"))

            # --- persistent SBUF tiles ---
            ones = sb.tile([1, 128], af, tag="ones")
            nc.gpsimd.memset(ones[:, :], 1.0)

            wc_sb = sb.tile([128, 2048], af, tag="wconv")
            bc_sb = sb.tile([1, 512], af, tag="bconv")

            refine = sb.tile([128, 4 * NT], af, tag="refine")
            y_sb = sb.tile([128, 5 * C], f32, tag="y")
            nc.gpsimd.memset(y_sb[64:, 4 * C:], 0.0)

            per = {}
            for pfx in ("sa", "ca"):
                t = {}
                t["b2"] = sb.tile([128, 6], f32, tag="b2" + pfx, name="b2_" + pfx)
                nc.sync.dma_start(t["b2"][:, :], bb2[pfx][:, :])
                t["w"] = sb.tile([128, 2048], af, tag="w" + pfx, name="w_" + pfx)
                nc.sync.dma_start(t["w"][:, :], wb[pfx][:, :])
                t["xq"] = sb.tile([128, 2 * NT], af, tag="xq" + pfx, name="xq_" + pfx)
                nc.sync.dma_start(t["xq"][:, :], xq[pfx][:, :])
                t["xkv"] = sb.tile([128, 2 * NKVP], af, tag="xkv" + pfx, name="xkv_" + pfx)
                for hh in range(2):
                    nc.sync.dma_start(
                        t["xkv"][:, hh * NKVP:(hh + 1) * NKVP],
                        xkv[pfx][:, hh * NKVP:(hh + 1) * NKVP])
                t["e"] = sb.tile([128, 6912], af, tag="e" + pfx, name="e_" + pfx)
                for hh in range(4):
                    nc.sync.dma_start(
                        t["e"][:, hh * 1728:(hh + 1) * 1728],
                        eb[pfx][:, hh * 1728:(hh + 1) * 1728])
                t["q"] = sb.tile([128, 2 * NT], af, tag="q" + pfx, name="q_" + pfx)
                t["k"] = sb.tile([128, 2 * NKVP], af, tag="k" + pfx, name="k_" + pfx)
                t["v"] = sb.tile([128, NCH * 512], af, tag="v" + pfx, name="v_" + pfx)
                # per-head ones columns (denominator rows of the AV matmul)
                on = t["v"][:, :].rearrange(
                    "p (c h e) -> p c h e", c=NCH, e=64)
                nc.gpsimd.memset(on[:, :, :, 32:], 1.0)
                t["attn"] = sb.tile([128, 2 * NT], af, tag="attn" + pfx, name="attn_" + pfx)
                per[pfx] = t

            nc.sync.dma_start(wc_sb[:, :], wconv[:, :])
            nc.sync.dma_start(bc_sb[:, :], bconv[:, :])

            def branch(pfx):
                t = per[pfx]
                xq_sb, xkv_sb, w_sb = t["xq"], t["xkv"], t["w"]
                b2_sb, e_sb = t["b2"], t["e"]
                q_sb, k_sb, v_sb, attn = t["q"], t["k"], t["v"], t["attn"]

                # --- Q projection: out chunk m, token block nb ---
                for m in range(2):
                    for nb in range(2):
                        ps = pp.tile([128, 512], f32, tag="proj")
                        for kc in range(2):
                            nc.tensor.matmul(
                                ps[:, :NQ],
                                mm(w_sb[:, kc * 256 + m * 128:
                                        kc * 256 + m * 128 + 128]),
                                mm(xq_sb[:, kc * NT + nb * NQ:
                                         kc * NT + nb * NQ + NQ]),
                                start=(kc == 0), stop=(kc == 1))
                        nc.scalar.activation(
                            q_sb[:, m * NT + nb * NQ:m * NT + nb * NQ + NQ],
                            ps[:, :NQ], mybir.ActivationFunctionType.Identity,
                            bias=b2_sb[:, m:m + 1])

                # --- K projection (blocks of 512,512,512,512,256 per half) ---
                for m in range(2):
                    for nb in range(5):
                        w0 = nb * 512
                        wn = min(512, NKVP - w0)
                        ps = pp.tile([128, 512], f32, tag="proj")
                        for kc in range(2):
                            nc.tensor.matmul(
                                ps[:, :wn],
                                mm(w_sb[:, 512 + kc * 512 + m * 128:
                                        512 + kc * 512 + m * 128 + 128]),
                                mm(xkv_sb[:, kc * NKVP + w0:
                                          kc * NKVP + w0 + wn]),
                                start=(kc == 0), stop=(kc == 1))
                        nc.scalar.activation(
                            k_sb[:, m * NKVP + w0:m * NKVP + w0 + wn],
                            ps[:, :wn], mybir.ActivationFunctionType.Identity,
                            bias=b2_sb[:, 2 + m:3 + m])

                # --- V projection (tokens on partitions, no bias) ---
                # two token chunks share one PSUM tile so the evacuation
                # copies 512 contiguous source columns per DVE op
                for cp in range(NCH // 2):
                    ps = pp.tile([128, 512], f32, tag="proj")
                    for half in range(2):
                        t0 = (2 * cp + half) * 128
                        for kc in range(2):
                            nc.tensor.matmul(
                                ps[:, half * 256:half * 256 + 256],
                                mm(xkv_sb[:, kc * NKVP + t0:
                                          kc * NKVP + t0 + 128]),
                                mm(w_sb[:, 512 + kc * 512 + 256:
                                        512 + kc * 512 + 512]),
                                start=(kc == 0), stop=(kc == 1))
                    dst = v_sb[:, cp * 1024:cp * 1024 + 1024]
                    dst = dst.rearrange("p (h e) -> p h e", e=64)[:, :, :32]
                    src_ = ps[:, :].rearrange("p (h e) -> p h e", e=32)
                    nc.vector.tensor_copy(dst, src_)

                # --- attention, software-pipelined over (b, h):
                # S(t)/exp(t) are emitted before AV(t-1) so the PE stream
                # never blocks behind the previous head's mask-multiply.
                def emit_av_div(st):
                    b, h, hp, hc, p_sb = st
                    av = ppAV.tile([64, 3 * NPQ], f32, tag="av")
                    for j in range(NP):
                        for cb in range(3):
                            idx = j * 3 + cb
                            cc = b * NP + j
                            off = (cc * 3 + cb) * 512 + 64 * h
                            nc.tensor.matmul(
                                av[:, j * NPQ:(j + 1) * NPQ],
                                mm(v_sb[:, off:off + 64]),
                                mm(p_sb[:, idx * NPQ:(idx + 1) * NPQ]),
                                start=(cb == 0), stop=(cb == 2))
                    # rows 32:63 hold the replicated softmax denominator
                    rec = sbV.tile([32, 3 * NPQ], f32, tag="rec")
                    nc.vector.reciprocal(rec[:, :], av[32:, :])
                    nc.vector.tensor_mul(
                        attn[hp:hp + 32, hc * NT + b * NQ:
                             hc * NT + b * NQ + NQ],
                        av[:32, :], rec[:, :])

                half = 0 if pfx == "sa" else 2

                def emit_oproj(nb):
                    for m in range(2):
                        ps = pp.tile([128, 512], f32, tag="proj")
                        for kc in range(2):
                            nc.tensor.matmul(
                                ps[:, :NQ],
                                mm(w_sb[:, 1536 + kc * 256 + m * 128:
                                        1536 + kc * 256 + m * 128 + 128]),
                                mm(attn[:, kc * NT + nb * NQ:
                                        kc * NT + nb * NQ + NQ]),
                                start=(kc == 0), stop=(kc == 1))
                        nc.scalar.activation(
                            refine[:, (half + m) * NT + nb * NQ:
                                   (half + m) * NT + nb * NQ + NQ],
                            ps[:, :NQ], mybir.ActivationFunctionType.Identity,
                            bias=b2_sb[:, 4 + m:5 + m])

                prev = None
                for b in range(B):
                    for h in range(HEADS):
                        hp = 32 * (h % 4)
                        hc = h // 4
                        psS = ppS.tile([128, 1024], f32, tag="s")
                        for j in range(NP):
                            for cb in range(3):
                                idx = j * 3 + cb
                                k0 = hc * NKVP + (b * NP + j) * WT + cb * 128
                                o = _ps_off(idx)
                                nc.tensor.matmul(
                                    psS[:, o:o + NPQ],
                                    mm(k_sb[hp:hp + 32, k0:k0 + 128]),
                                    mm(q_sb[hp:hp + 32,
                                            hc * NT + b * NQ + j * NPQ:
                                            hc * NT + b * NQ + (j + 1) * NPQ]),
                                    start=True, stop=True,
                                    tile_position=(hp, 0))
                        p_sb = sbP.tile([128, 9 * NPQ], af, tag="p")
                        nc.scalar.activation(
                            p_sb[:, :480], psS[:, :480],
                            mybir.ActivationFunctionType.Exp, scale=SCALE)
                        nc.scalar.activation(
                            p_sb[:, 480:], psS[:, 512:896],
                            mybir.ActivationFunctionType.Exp, scale=SCALE)
                        if prev is not None:
                            emit_av_div(prev)
                        if h % 2 == 0:
                            nc.vector.tensor_mul(
                                p_sb[:, :624], p_sb[:, :624],
                                e_sb[:, h * 864:h * 864 + 624])
                            nc.gpsimd.tensor_tensor(
                                p_sb[:, 624:], p_sb[:, 624:],
                                e_sb[:, h * 864 + 624:h * 864 + 864],
                                mybir.AluOpType.mult)
                        else:
                            nc.gpsimd.tensor_tensor(
                                p_sb[:, :], p_sb[:, :],
                                e_sb[:, h * 864:h * 864 + 864],
                                mybir.AluOpType.mult)
                        prev = (b, h, hp, hc, p_sb)
                    if b == 0:
                        emit_av_div(prev)
                        prev = None
                        emit_oproj(0)
                emit_av_div(prev)
                emit_oproj(1)

            branch("sa")
            branch("ca")

            # --- conv + BN + ReLU ---
            for mt in range(5):
                ntok = 128 if mt < 4 else 64
                ps = pp.tile([128, 512], f32, tag="proj")
                for kc in range(4):
                    nc.tensor.matmul(
                        ps[:ntok, :],
                        mm(refine[:, kc * NT + mt * 128:
                                  kc * NT + mt * 128 + ntok]),
                        mm(wc_sb[:, kc * 512:kc * 512 + 512]),
                        start=(kc == 0), stop=False)
                nc.tensor.matmul(
                    ps[:ntok, :], mm(ones[:, :ntok]), mm(bc_sb[:, :]),
                    start=False, stop=True)
                nc.scalar.activation(
                    y_sb[:ntok, mt * 512:mt * 512 + 512], ps[:ntok, :],
                    mybir.ActivationFunctionType.Relu)
                nc.sync.dma_start(
                    y_out[:, mt * 512:mt * 512 + 512],
                    y_sb[:, mt * 512:mt * 512 + 512])

    if split_waits:
        _split_waits(nc, mybir)
    return nc


def _split_waits(nc, mybir):
    """walrus in this container accepts at most ONE sync-wait per
    instruction; move extra waits onto injected same-engine NOPs."""
    import bass_rust
    nid = [0]
    for fn in nc.m.functions:
        for bb in fn.blocks:
            out = []
            for inst in bb.instructions:
                si = inst.sync_info
                if si is not None and len(si.on_wait) > 1:
                    waits = list(si.on_wait)
                    for wv in waits[:-1]:
                        nid[0] += 1
                        nop = bass_rust.InstNoOp(
                            name=f"WSPLIT-{nid[0]}", ins=[], outs=[])
                        nop.engine = inst.engine
                        nop.sync_info = mybir.SyncInfo(
                            on_wait=[wv], on_update=[])
                        out.append(nop)
                    inst.sync_info = mybir.SyncInfo(
                        on_wait=[waits[-1]], on_update=list(si.on_update))
                out.append(inst)
            bb.instructions[:] = out


def kernel(**inputs):
    from concourse import bass_utils

    import time as _time
    inputs = {k: np.asarray(v) for k, v in inputs.items()}
    mm_dtype = os.environ.get("MM_DTYPE", "bfloat16")
    nc = build_nc(mm_dtype)
    in_maps = [_prep_core(inputs, c) for c in range(N_CORES)]
    t0 = _time.perf_counter()
    res = bass_utils.run_bass_kernel_spmd(
        nc, in_maps, core_ids=list(range(N_CORES)))
    t1 = _time.perf_counter()
    if res.exec_time_ns:
        print(f"HW exec time: {res.exec_time_ns} ns")
    print(f"[kernel] spmd call wall: {(t1 - t0) * 1e3:.1f} ms")

    out = np.zeros((B, C, HS, HS), dtype=F32)
    for c in range(N_CORES):
        y = res.results[c]["y"]  # [128, 2560]
        y = y.reshape(128, 5, C).transpose(1, 0, 2).reshape(640, C)[:NT]
        q_lo = NX * c
        out[:, :, q_lo:q_lo + NX, :] = \
            y.reshape(B, NX, HS, C).transpose(0, 3, 1, 2)
    return out
